# revision 24
# baseline (speedup 1.0000x reference)
"""Trainium2 Bass kernel for nn_CellularAutomatonDecoder.

Model (per reference):
  cells = embed[tokens] + pos_embed                        (B, T, D)
  rule_bias MLP from mean(c_states); const_bias = rule_bias @ W1b + b1
  8x CA steps: pre = cells@W1c + roll(cells,+1)@W1l + roll(cells,-1)@W1r + const_bias
               cells = a*cells + (1-a)*tanh(gelu(pre) @ W2 + b2)
  out = LN(cells) @ head_w                                 (B, T, V)

Sharding: pure data-parallel over batch across 8 cores (256 rows each).

Device design notes:
- feature-major state sigma[d=128 partitions, 8192 tokens] in SBUF, token
  order t-major (col j = t*256 + b_local): the T-axis roll becomes a +-256
  column shift with one wrap piece -> every matmul is contiguous with
  N>=256, where fp32r streams at ~1 cycle/row.
- state kept scaled: sigma = cells/(1-a); the leaky blend is one fused DVE
  op sigma' = a*sigma + tanh(...); (1-a) folds into activation scales and
  host-prescaled weights.
- embedding gather = one-hot matmul (one-hot built on DVE from a K=1
  token-broadcast matmul + iota-column compare).
- head uses sigma blocks as the stationary operand so output lands
  token-major in PSUM and DMAs out contiguously; LayerNorm reduces to a
  per-token inv-std scale (ln_g and mean-removal folded into a
  column-centered head weight matrix on the host).
- LN stats via ones-matmuls -> PE micro-transposes -> token-major [128,64]
  tiles, so all per-token scalar math runs across partitions.
- single PSUM pool with shared tags across init/evolve/final: no pool
  release barriers between phases (keeps PE dense, avoids HAM re-throttle).
"""

import os
import sys

import numpy as np

for _p in ("/opt/trn_rl_repo", "/root/.axon_site/_ro/trn_rl_repo"):
    if os.path.isdir(_p) and _p not in sys.path:
        sys.path.append(_p)

from contextlib import ExitStack

import concourse.bacc as bacc
import concourse.tile as tile
from concourse import mybir
from concourse.bass_utils import run_bass_kernel_spmd

F32 = mybir.dt.float32
F32R = mybir.dt.float32r
AF = mybir.ActivationFunctionType
ALU = mybir.AluOpType
AX = mybir.AxisListType

B, T, D, V, CDIM = 2048, 32, 128, 256, 128
NEV = 8
EPS = 1e-5
NC = 8
BL = B // NC          # 256 batch rows per core
NTOK = BL * T         # 8192 tokens per core
CH = 1024             # token chunk (columns)
NCH = NTOK // CH      # 8 chunks
NBLK = NTOK // 128    # 64 head blocks

TRACE = False         # test harness may flip this (with prof shim installed)
_CACHE = {}


def _pieces(dst0, n, shift):
    """Contiguous (dst, src, len) pieces of src = (dst + shift) mod NTOK."""
    out = []
    j = 0
    while j < n:
        s = (dst0 + j + shift) % NTOK
        ln = min(n - j, NTOK - s)
        out.append((dst0 + j, s, ln))
        j += ln
    return out


def _build(a, has_lnb):
    ia = 1.0 - a
    nc = bacc.Bacc("TRN2", target_bir_lowering=False, debug=False, num_devices=NC)

    tok_d = nc.dram_tensor("tok", [1, NTOK], F32R, kind="ExternalInput").ap()
    spack_d = nc.dram_tensor("spack", [128, 128], F32R, kind="ExternalInput").ap()
    cpack_d = nc.dram_tensor("cpack", [128, 46], F32, kind="ExternalInput").ap()
    epack_d = nc.dram_tensor("epack", [128, 256], F32R, kind="ExternalInput").ap()
    wpack_d = nc.dram_tensor("wpack", [128, 1280], F32R, kind="ExternalInput").ap()
    fpack_d = nc.dram_tensor("fpack", [128, 1024], F32, kind="ExternalInput").ap()
    out_d = nc.dram_tensor("out", [NTOK, V], F32, kind="ExternalOutput").ap()
    out_r = out_d.rearrange("(b t) v -> b t v", t=T)

    with tile.TileContext(nc) as tc, ExitStack() as ctx:
        # ---- persistent SBUF ----
        wpool = ctx.enter_context(tc.tile_pool(name="weights", bufs=1))
        spack = wpool.tile([128, 128], F32R, tag="spack")
        nc.sync.dma_start(spack[:], spack_d)
        cpack = wpool.tile([128, 46], F32, tag="cpack")
        nc.sync.dma_start(cpack[:], cpack_d)
        epack = wpool.tile([128, 256], F32R, tag="epack")
        nc.sync.dma_start(epack[:], epack_d)
        wpack = wpool.tile([128, 1280], F32R, tag="wpack")
        nc.sync.dma_start(wpack[:], wpack_d)
        fpack = wpool.tile([128, 1024], F32, tag="fpack")
        nc.sync.dma_start(fpack[:], fpack_d)

        ones_s = spack
        emb_s = epack[:, 0:256]
        wc_s, wl_s, wr_s = wpack[:, 0:256], wpack[:, 256:512], wpack[:, 512:768]
        w2_s, hwc_s = wpack[:, 768:1024], wpack[:, 1024:1280]
        w1b_s, wc1_s, wc2_s = fpack[:, 0:256], fpack[:, 256:512], fpack[:, 512:768]
        consth_r = fpack[0:1, 768:1024].bitcast(F32R)
        posT_s, cT_s = cpack[:, 0:32], cpack[:, 32:36]
        bc1_s, bc2_s = cpack[:, 36:38], cpack[:, 38:39]
        b1_s, b2_s = cpack[:, 39:41], cpack[:, 41:42]
        vid_s = cpack[:, 42:44]
        eye2_s = cpack[0:2, 44:46]

        spool = ctx.enter_context(tc.tile_pool(name="state", bufs=1))
        sig = spool.tile([128, NTOK], F32R, tag="sigma")
        stats_tm = spool.tile([128, 2 * NBLK], F32, tag="stats_tm")

        mlp_sb = ctx.enter_context(tc.tile_pool(name="mlp_sb", bufs=1))
        cbias_s = mlp_sb.tile([128, 2], F32, tag="cbias")

        # shared pools, all phases (no release barriers)
        pp = ctx.enter_context(tc.tile_pool(name="psum", bufs=1, space="PSUM"))
        # tag "pre": 3 slots x [128,1024] (2 banks each); tag "new": 1 slot
        sbh = ctx.enter_context(tc.tile_pool(name="h_sb", bufs=4))
        sbt = ctx.enter_context(tc.tile_pool(name="t_sb", bufs=NCH + 1))
        sbtok = ctx.enter_context(tc.tile_pool(name="tok_sb", bufs=4))
        sbsr = ctx.enter_context(tc.tile_pool(name="srow_sb", bufs=3))
        sbst = ctx.enter_context(tc.tile_pool(name="stat_sb", bufs=1))
        sbo = ctx.enter_context(tc.tile_pool(name="out_sb", bufs=4))

        def ptile(shape, tag, name):
            return pp.tile(shape, F32, tag=tag, name=name, bufs=3 if tag == "pre" else 1)

        # ---- init: token gather via one-hot matmuls ----
        tok_tiles = {}
        for ci in [(NCH - 1 + j) % NCH for j in range(NCH)]:
            c0 = ci * CH
            tok_t = sbtok.tile([1, CH], F32R, tag="tok", name="tok_t")
            nc.sync.dma_start(tok_t[:], tok_d[0:1, c0:c0 + CH])
            tok_tiles[ci] = tok_t
        for ci in [(NCH - 1 + j) % NCH for j in range(NCH)]:
            c0 = ci * CH
            tok_t = tok_tiles[ci]
            oh_lo = sbh.tile([128, CH], F32R, tag="h", name="oh_lo")
            oh_hi = sbh.tile([128, CH], F32R, tag="h", name="oh_hi")
            for k in range(2):
                jc = slice(k * 512, (k + 1) * 512)
                tb_ps = ptile([128, 512], "new" if k == 0 else "pre", "tb_ps")
                nc.tensor.matmul(tb_ps[:], ones_s[0:1, 0:128], tok_t[0:1, jc],
                                 start=True, stop=True)
                nc.vector.tensor_scalar(oh_lo[:, jc], tb_ps[:], vid_s[:, 0:1], None,
                                        ALU.is_equal)
                nc.vector.tensor_scalar(oh_hi[:, jc], tb_ps[:], vid_s[:, 1:2], None,
                                        ALU.is_equal)
            cells_ps = ptile([128, CH], "pre", "cells_ps")
            for k in range(2):
                jc = slice(k * 512, (k + 1) * 512)
                nc.tensor.matmul(cells_ps[:, jc], emb_s[:, 0:128], oh_lo[:, jc],
                                 start=True, stop=False)
                nc.tensor.matmul(cells_ps[:, jc], emb_s[:, 128:256], oh_hi[:, jc],
                                 start=False, stop=True)
            for kb in range(CH // 256):
                tt = (c0 + kb * 256) // 256  # col j = t*256 + b -> t = j//256
                nc.scalar.activation(sig[:, c0 + kb * 256: c0 + (kb + 1) * 256],
                                     cells_ps[:, kb * 256:(kb + 1) * 256],
                                     AF.Identity, bias=posT_s[:, tt:tt + 1])

        # ---- rule-bias MLP (tiny; overlaps gather) ----
        cp_s = mlp_sb.tile([128, 1], F32, tag="cp")
        nc.vector.tensor_reduce(cp_s[:], cT_s[:], axis=AX.X, op=ALU.add)
        y1_ps = ptile([128, 2], "new", "y1_ps")
        for h in range(2):
            nc.tensor.matmul(y1_ps[:, h:h + 1], wc1_s[:, h * 128:(h + 1) * 128],
                             cp_s[:], start=True, stop=True)
        y1g_s = mlp_sb.tile([128, 2], F32, tag="y1g")
        for h in range(2):
            nc.scalar.activation(y1g_s[:, h:h + 1], y1_ps[:, h:h + 1], AF.Gelu,
                                 bias=bc1_s[:, h:h + 1], scale=0.25)
        rb_ps = ptile([128, 2], "new", "rb_ps")
        nc.tensor.matmul(rb_ps[:, 0:1], wc2_s[:, 0:128], y1g_s[:, 0:1],
                         start=True, stop=False)
        nc.tensor.matmul(rb_ps[:, 0:1], wc2_s[:, 128:256], y1g_s[:, 1:2],
                         start=False, stop=True)
        rb_s = mlp_sb.tile([128, 1], F32, tag="rb")
        nc.scalar.activation(rb_s[:], rb_ps[:, 0:1], AF.Identity, bias=bc2_s[:, 0:1])
        cb_ps = ptile([128, 2], "new", "cb_ps")
        for h in range(2):
            nc.tensor.matmul(cb_ps[:, h:h + 1], w1b_s[:, h * 128:(h + 1) * 128],
                             rb_s[:], start=True, stop=True)
        for h in range(2):
            nc.scalar.activation(cbias_s[:, h:h + 1], cb_ps[:, h:h + 1], AF.Identity,
                                 bias=b1_s[:, h:h + 1])

        # ---- evolve: 8 CA steps ----
        def emit_chunk(ci, s):
            c0 = ci * CH
            pre = [ptile([128, CH], "pre", f"pre{h_}") for h_ in range(2)]
            for h in range(2):
                hcols = slice(h * 128, (h + 1) * 128)
                for k in range(2):
                    d0 = c0 + k * 512
                    segs = [(wc_s, [(d0, d0, 512)]),
                            (wl_s, _pieces(d0, 512, -256)),
                            (wr_s, _pieces(d0, 512, +256))]
                    flat = [(w, dd, ss, ll) for w, ps in segs for dd, ss, ll in ps]
                    for i, (w, dd, ss, ll) in enumerate(flat):
                        nc.tensor.matmul(
                            pre[h][:, dd - c0: dd - c0 + ll],
                            w[:, hcols], sig[:, ss:ss + ll],
                            start=(i == 0), stop=(i == len(flat) - 1))
            h_t = [sbh.tile([128, CH], F32R, tag="h", name=f"ht{h_}")
                   for h_ in range(2)]
            for h in range(2):
                nc.scalar.activation(h_t[h][:], pre[h][:], AF.Gelu,
                                     bias=cbias_s[:, h:h + 1], scale=ia)
            new_ps = ptile([128, CH], "new", "new_ps")
            for k in range(2):
                jc = slice(k * 512, (k + 1) * 512)
                nc.tensor.matmul(new_ps[:, jc], w2_s[:, 0:128], h_t[0][:, jc],
                                 start=True, stop=False)
                nc.tensor.matmul(new_ps[:, jc], w2_s[:, 128:256], h_t[1][:, jc],
                                 start=False, stop=True)
            t_t = sbt.tile([128, CH], F32, tag="t", name="t_t")
            nc.scalar.activation(t_t[:], new_ps[:], AF.Tanh, bias=b2_s[:, 0:1])
            return t_t

        def emit_blend(ci, t_t):
            c0 = ci * CH
            nc.vector.scalar_tensor_tensor(
                sig[:, c0:c0 + CH], sig[:, c0:c0 + CH], a, t_t[:],
                op0=ALU.mult, op1=ALU.add)

        def emit_stats(ci):
            c0 = ci * CH
            sq_t = sbh.tile([128, CH], F32R, tag="h", name="sq_t")
            nc.scalar.activation(sq_t[:], sig[:, c0:c0 + CH], AF.Square)
            sr1 = ptile([1, CH], "pre", "sr1")
            sr2 = ptile([1, CH], "pre", "sr2")
            for k in range(2):
                jc = slice(k * 512, (k + 1) * 512)
                nc.tensor.matmul(sr1[0:1, jc], ones_s[:, 0:1],
                                 sig[:, c0 + k * 512:c0 + (k + 1) * 512],
                                 start=True, stop=True)
                nc.tensor.matmul(sr2[0:1, jc], ones_s[:, 0:1], sq_t[:, jc],
                                 start=True, stop=True)
            srow_t = sbsr.tile([1, 2 * CH], F32, tag="srow", name="srow_t")
            nc.scalar.activation(srow_t[0:1, 0:CH], sr1[0:1, :], AF.Copy)
            nc.vector.tensor_copy(srow_t[0:1, CH:2 * CH], sr2[0:1, :])
            stp = ptile([128, 16], "new", "stp")
            for j in range(CH // 128):
                nc.tensor.transpose(stp[:, 2 * j:2 * j + 1],
                                    srow_t[0:1, j * 128:(j + 1) * 128],
                                    eye2_s[0:1, 0:1])
                nc.tensor.transpose(stp[:, 2 * j + 1:2 * j + 2],
                                    srow_t[0:1, CH + j * 128:CH + (j + 1) * 128],
                                    eye2_s[0:1, 0:1])
            nc.vector.tensor_copy(stats_tm[:, 16 * ci:16 * (ci + 1)], stp[:])

        for s in range(NEV - 1):
            order = [(s + j) % NCH for j in range(NCH)]
            t_tiles = {}
            for ci in order:
                t_tiles[ci] = emit_chunk(ci, s)
            for ci in order:
                emit_blend(ci, t_tiles[ci])

        # last step: blends lag chunk processing by 2 (a blend only needs its
        # own and both neighbor chunks' matmuls done); each chunk's LN stats
        # follow its blend immediately, overlapping the rest of the step
        P = [(NCH - 2 + j) % NCH for j in range(NCH)]
        t7 = {}
        warm_s = sbst.tile([1, 8], F32, tag="warm")
        nc.scalar.activation(warm_s[:], cpack[0:1, 0:8], AF.Sqrt)
        for i, ci in enumerate(P):
            t7[ci] = emit_chunk(ci, NEV - 1)
            if i >= 2:
                emit_blend(P[i - 1], t7[P[i - 1]])
                emit_stats(P[i - 1])
        for ci in (P[7], P[0]):
            emit_blend(ci, t7[ci])
            emit_stats(ci)

        # ---- final: per-token inv-std + head ----
        st3 = stats_tm[:].rearrange("p (b two) -> p b two", two=2)
        s1ap = st3[:, :, 0]
        s2ap = st3[:, :, 1]
        m2_s = sbst.tile([128, NBLK], F32, tag="m2")
        nc.scalar.activation(m2_s[:], s1ap, AF.Square, scale=ia / 128.0)
        vf_s = sbst.tile([128, NBLK], F32, tag="vf")
        nc.vector.scalar_tensor_tensor(vf_s[:], s2ap, ia * ia / 128.0, m2_s[:],
                                       op0=ALU.mult, op1=ALU.subtract)
        nc.vector.tensor_scalar_add(vf_s[:], vf_s[:], EPS)
        sd_s = sbst.tile([128, NBLK], F32, tag="sd")
        nc.scalar.activation(sd_s[:], vf_s[:], AF.Sqrt)
        y0_s = sbst.tile([128, NBLK], F32, tag="y0")
        nc.vector.reciprocal(y0_s[:], sd_s[:])
        q_s = sbst.tile([128, NBLK], F32, tag="q")
        nc.vector.tensor_mul(q_s[:], y0_s[:], y0_s[:])
        w_s = sbst.tile([128, NBLK], F32, tag="w")
        nc.vector.scalar_tensor_tensor(w_s[:], vf_s[:], -0.5, q_s[:],
                                       op0=ALU.mult, op1=ALU.mult)
        inv_s = sbst.tile([128, NBLK], F32, tag="inv")
        nc.vector.scalar_tensor_tensor(inv_s[:], w_s[:], 1.5, y0_s[:],
                                       op0=ALU.add, op1=ALU.mult)
        # keep the PE busy through the col-math + tail-stats window so the
        # clock monitor stays at full rate into the head loop
        for dwi in range(30):
            dummy_ps = ptile([128, 512], "new", "dummy_ps")
            nc.tensor.matmul(dummy_ps[:], wc_s[:, 0:128],
                             sig[:, (dwi % 16) * 512:(dwi % 16) * 512 + 512],
                             start=True, stop=True)
        # head: out[tok, v] = inv[tok] * (sigma_blk^T @ hwc)
        for b in range(NBLK):
            A_ps = ptile([128, V], "pre", "A_ps")
            nc.tensor.matmul(A_ps[:], sig[:, b * 128:(b + 1) * 128], hwc_s[:],
                             start=True, stop=True)
            o_t = sbo.tile([128, V], F32, tag="o", name="o_t")
            if b % 2 == 0:
                nc.vector.tensor_scalar(o_t[:], A_ps[:], inv_s[:, b:b + 1], None,
                                        ALU.mult)
            else:
                nc.scalar.activation(o_t[:], A_ps[:], AF.Copy,
                                     scale=inv_s[:, b:b + 1])
            tt = b // 2
            b0 = (b % 2) * 128
            eng = nc.sync if b % 2 == 0 else nc.gpsimd
            eng.dma_start(out_r[b0:b0 + 128, tt, :], o_t[:])

    nc.compile()
    return nc


def kernel(**inputs):
    g = {k: np.asarray(v, np.float32) if k != "tokens" else np.asarray(v)
         for k, v in inputs.items()}
    alpha = float(g["alpha"])
    a = float(1.0 / (1.0 + np.exp(-np.float64(alpha))))
    ia = 1.0 - a
    ln_b = g["ln_b"]
    has_lnb = bool(np.any(ln_b != 0))
    key = (np.float64(a).tobytes(), has_lnb)
    if key not in _CACHE:
        _CACHE[key] = _build(a, has_lnb)
    nc = _CACHE[key]

    W1, W2 = g["W1"], g["W2"]
    embed, pos = g["embed"], g["pos_embed"]
    head_w, ln_g = g["head_w"], g["ln_g"]

    spack = np.ones((128, 128), np.float32)

    cpack = np.zeros((128, 46), np.float32)
    cpack[:, 0:32] = pos.T * np.float32(1.0 / ia)
    cpack[:, 32:36] = g["c_states"].T
    cpack[:, 36:38] = g["bc1"].reshape(2, 128).T
    cpack[:, 38:39] = g["bc2"].reshape(128, 1)
    cpack[:, 39:41] = g["b1"].reshape(2, 128).T
    cpack[:, 41:42] = g["b2"].reshape(128, 1)
    cpack[:, 42:44] = np.stack([np.arange(128), np.arange(128, 256)], axis=1)
    cpack[0:2, 44:46] = np.eye(2, dtype=np.float32)

    epack = np.concatenate([embed[0:128], embed[128:256]],
                           axis=1) * np.float32(1.0 / ia)
    wpack = np.zeros((128, 1280), np.float32)
    wpack[:, 0:256] = W1[0:128]
    wpack[:, 256:512] = W1[128:256]
    wpack[:, 512:768] = W1[256:384]
    wpack[:, 768:1024] = np.concatenate([W2[0:128], W2[128:256]], axis=1)
    ghw = head_w * ln_g[:, None]
    wpack[:, 1024:1280] = (ghw - ghw.mean(axis=0, keepdims=True)) * np.float32(ia)

    fpack = np.zeros((128, 1024), np.float32)
    fpack[:, 0:256] = W1[384:512]
    fpack[:, 256:512] = g["Wc1"]
    fpack[:, 512:768] = np.concatenate([g["Wc2"][0:128], g["Wc2"][128:256]], axis=1)
    if has_lnb:
        fpack[0:1, 768:1024] = (ln_b @ head_w).reshape(1, V)

    tokens = g["tokens"]
    in_maps = []
    for c in range(NC):
        tk = tokens[c * BL:(c + 1) * BL].astype(np.float32)   # (BL, T)
        in_maps.append({
            "tok": np.ascontiguousarray(tk.T).reshape(1, NTOK),  # t-major
            "spack": spack, "cpack": cpack, "epack": epack,
            "wpack": wpack, "fpack": fpack,
        })

    kw = {}
    if TRACE:
        kw = dict(trace=True)
    res = run_bass_kernel_spmd(nc, in_maps, core_ids=list(range(NC)), **kw)
    if TRACE and res.exec_time_ns is not None:
        print(f"HW exec time: {res.exec_time_ns} ns")
        kernel.last_exec_ns = res.exec_time_ns
        kernel.last_trace = res.instructions_and_trace
    out = np.stack([res.results[c]["out"] for c in range(NC)], axis=0)
    out = out.reshape(B, T, V)
    if has_lnb:
        out = out + (ln_b @ head_w)[None, None, :]
    return np.ascontiguousarray(out)


# revision 25
# speedup vs baseline: 1.0491x; 1.0491x over previous
"""Trainium2 Bass kernel for nn_CellularAutomatonDecoder.

Model (per reference):
  cells = embed[tokens] + pos_embed                        (B, T, D)
  rule_bias MLP from mean(c_states); const_bias = rule_bias @ W1b + b1
  8x CA steps: pre = cells@W1c + roll(cells,+1)@W1l + roll(cells,-1)@W1r + const_bias
               cells = a*cells + (1-a)*tanh(gelu(pre) @ W2 + b2)
  out = LN(cells) @ head_w                                 (B, T, V)

Sharding: pure data-parallel over batch across 8 cores (256 rows each).

Device design notes:
- feature-major state sigma[d=128 partitions, 8192 tokens] in SBUF, token
  order t-major (col j = t*256 + b_local): the T-axis roll becomes a +-256
  column shift with one wrap piece -> every matmul is contiguous with
  N>=256, where fp32r streams at ~1 cycle/row.
- state kept scaled: sigma = cells/(1-a); the leaky blend is one fused DVE
  op sigma' = a*sigma + tanh(...); (1-a) folds into activation scales and
  host-prescaled weights.
- embedding gather = one-hot matmul (one-hot built on DVE from a K=1
  token-broadcast matmul + iota-column compare).
- head uses sigma blocks as the stationary operand so output lands
  token-major in PSUM and DMAs out contiguously; LayerNorm reduces to a
  per-token inv-std scale (ln_g and mean-removal folded into a
  column-centered head weight matrix on the host).
- LN stats via ones-matmuls -> PE micro-transposes -> token-major [128,64]
  tiles, so all per-token scalar math runs across partitions.
- single PSUM pool with shared tags across init/evolve/final: no pool
  release barriers between phases (keeps PE dense, avoids HAM re-throttle).
"""

import os
import sys

import numpy as np

for _p in ("/opt/trn_rl_repo", "/root/.axon_site/_ro/trn_rl_repo"):
    if os.path.isdir(_p) and _p not in sys.path:
        sys.path.append(_p)

from contextlib import ExitStack

import concourse.bacc as bacc
import concourse.tile as tile
from concourse import mybir
from concourse.bass_utils import run_bass_kernel_spmd

F32 = mybir.dt.float32
F32R = mybir.dt.float32r
AF = mybir.ActivationFunctionType
ALU = mybir.AluOpType
AX = mybir.AxisListType

B, T, D, V, CDIM = 2048, 32, 128, 256, 128
NEV = 8
EPS = 1e-5
NC = 8
BL = B // NC          # 256 batch rows per core
NTOK = BL * T         # 8192 tokens per core
CH = 1024             # token chunk (columns)
NCH = NTOK // CH      # 8 chunks
NBLK = NTOK // 128    # 64 head blocks

TRACE = False         # test harness may flip this (with prof shim installed)
_CACHE = {}


def _pieces(dst0, n, shift):
    """Contiguous (dst, src, len) pieces of src = (dst + shift) mod NTOK."""
    out = []
    j = 0
    while j < n:
        s = (dst0 + j + shift) % NTOK
        ln = min(n - j, NTOK - s)
        out.append((dst0 + j, s, ln))
        j += ln
    return out


def _build(a, has_lnb):
    ia = 1.0 - a
    nc = bacc.Bacc("TRN2", target_bir_lowering=False, debug=False, num_devices=NC)

    tok_d = nc.dram_tensor("tok", [1, NTOK], F32R, kind="ExternalInput").ap()
    spack_d = nc.dram_tensor("spack", [128, 128], F32R, kind="ExternalInput").ap()
    cpack_d = nc.dram_tensor("cpack", [128, 46], F32, kind="ExternalInput").ap()
    epack_d = nc.dram_tensor("epack", [128, 256], F32R, kind="ExternalInput").ap()
    wpack_d = nc.dram_tensor("wpack", [128, 1280], F32R, kind="ExternalInput").ap()
    fpack_d = nc.dram_tensor("fpack", [128, 1024], F32, kind="ExternalInput").ap()
    out_d = nc.dram_tensor("out", [NTOK, V], F32, kind="ExternalOutput").ap()
    out_r = out_d.rearrange("(b t) v -> b t v", t=T)

    with tile.TileContext(nc) as tc, ExitStack() as ctx:
        # ---- persistent SBUF ----
        wpool = ctx.enter_context(tc.tile_pool(name="weights", bufs=1))
        spack = wpool.tile([128, 128], F32R, tag="spack")
        nc.sync.dma_start(spack[:], spack_d)
        cpack = wpool.tile([128, 46], F32, tag="cpack")
        nc.sync.dma_start(cpack[:], cpack_d)
        epack = wpool.tile([128, 256], F32R, tag="epack")
        nc.sync.dma_start(epack[:], epack_d)
        wpack = wpool.tile([128, 1280], F32R, tag="wpack")
        nc.sync.dma_start(wpack[:], wpack_d)
        fpack = wpool.tile([128, 1024], F32, tag="fpack")
        nc.sync.dma_start(fpack[:], fpack_d)

        ones_s = spack
        emb_s = epack[:, 0:256]
        wc_s, wl_s, wr_s = wpack[:, 0:256], wpack[:, 256:512], wpack[:, 512:768]
        w2_s, hwc_s = wpack[:, 768:1024], wpack[:, 1024:1280]
        w1b_s, wc1_s, wc2_s = fpack[:, 0:256], fpack[:, 256:512], fpack[:, 512:768]
        consth_r = fpack[0:1, 768:1024].bitcast(F32R)
        posT_s, cT_s = cpack[:, 0:32], cpack[:, 32:36]
        bc1_s, bc2_s = cpack[:, 36:38], cpack[:, 38:39]
        b1_s, b2_s = cpack[:, 39:41], cpack[:, 41:42]
        vid_s = cpack[:, 42:44]
        eye2_s = cpack[0:2, 44:46]

        spool = ctx.enter_context(tc.tile_pool(name="state", bufs=1))
        sig = spool.tile([128, NTOK], F32R, tag="sigma")
        stats_tm = spool.tile([128, 2 * NBLK], F32, tag="stats_tm")

        mlp_sb = ctx.enter_context(tc.tile_pool(name="mlp_sb", bufs=1))
        cbias_s = mlp_sb.tile([128, 2], F32, tag="cbias")

        # shared pools, all phases (no release barriers)
        pp = ctx.enter_context(tc.tile_pool(name="psum", bufs=1, space="PSUM"))
        # tag "pre": 3 slots x [128,1024] (2 banks each); tag "new": 1 slot
        sbh = ctx.enter_context(tc.tile_pool(name="h_sb", bufs=4))
        sbt = ctx.enter_context(tc.tile_pool(name="t_sb", bufs=NCH + 1))
        sbtok = ctx.enter_context(tc.tile_pool(name="tok_sb", bufs=4))
        sbsr = ctx.enter_context(tc.tile_pool(name="srow_sb", bufs=3))
        sbst = ctx.enter_context(tc.tile_pool(name="stat_sb", bufs=1))
        sbo = ctx.enter_context(tc.tile_pool(name="out_sb", bufs=4))

        def ptile(shape, tag, name):
            return pp.tile(shape, F32, tag=tag, name=name, bufs=3 if tag == "pre" else 1)

        # ---- init: token gather via one-hot matmuls ----
        tok_tiles = {}
        for ci in [(NCH - 1 + j) % NCH for j in range(NCH)]:
            c0 = ci * CH
            tok_t = sbtok.tile([1, CH], F32R, tag="tok", name="tok_t")
            nc.sync.dma_start(tok_t[:], tok_d[0:1, c0:c0 + CH])
            tok_tiles[ci] = tok_t
        for ci in [(NCH - 1 + j) % NCH for j in range(NCH)]:
            c0 = ci * CH
            tok_t = tok_tiles[ci]
            oh_lo = sbh.tile([128, CH], F32R, tag="h", name="oh_lo")
            oh_hi = sbh.tile([128, CH], F32R, tag="h", name="oh_hi")
            for k in range(2):
                jc = slice(k * 512, (k + 1) * 512)
                tb_ps = ptile([128, 512], "new" if k == 0 else "pre", "tb_ps")
                nc.tensor.matmul(tb_ps[:], ones_s[0:1, 0:128], tok_t[0:1, jc],
                                 start=True, stop=True)
                nc.vector.tensor_scalar(oh_lo[:, jc], tb_ps[:], vid_s[:, 0:1], None,
                                        ALU.is_equal)
                nc.vector.tensor_scalar(oh_hi[:, jc], tb_ps[:], vid_s[:, 1:2], None,
                                        ALU.is_equal)
            cells_ps = ptile([128, CH], "pre", "cells_ps")
            for k in range(2):
                jc = slice(k * 512, (k + 1) * 512)
                nc.tensor.matmul(cells_ps[:, jc], emb_s[:, 0:128], oh_lo[:, jc],
                                 start=True, stop=False)
                nc.tensor.matmul(cells_ps[:, jc], emb_s[:, 128:256], oh_hi[:, jc],
                                 start=False, stop=True)
            for kb in range(CH // 256):
                tt = (c0 + kb * 256) // 256  # col j = t*256 + b -> t = j//256
                nc.scalar.activation(sig[:, c0 + kb * 256: c0 + (kb + 1) * 256],
                                     cells_ps[:, kb * 256:(kb + 1) * 256],
                                     AF.Identity, bias=posT_s[:, tt:tt + 1])

        # ---- rule-bias MLP (tiny; overlaps gather) ----
        cp_s = mlp_sb.tile([128, 1], F32, tag="cp")
        nc.vector.tensor_reduce(cp_s[:], cT_s[:], axis=AX.X, op=ALU.add)
        y1_ps = ptile([128, 2], "new", "y1_ps")
        for h in range(2):
            nc.tensor.matmul(y1_ps[:, h:h + 1], wc1_s[:, h * 128:(h + 1) * 128],
                             cp_s[:], start=True, stop=True)
        y1g_s = mlp_sb.tile([128, 2], F32, tag="y1g")
        for h in range(2):
            nc.scalar.activation(y1g_s[:, h:h + 1], y1_ps[:, h:h + 1], AF.Gelu,
                                 bias=bc1_s[:, h:h + 1], scale=0.25)
        rb_ps = ptile([128, 2], "new", "rb_ps")
        nc.tensor.matmul(rb_ps[:, 0:1], wc2_s[:, 0:128], y1g_s[:, 0:1],
                         start=True, stop=False)
        nc.tensor.matmul(rb_ps[:, 0:1], wc2_s[:, 128:256], y1g_s[:, 1:2],
                         start=False, stop=True)
        rb_s = mlp_sb.tile([128, 1], F32, tag="rb")
        nc.scalar.activation(rb_s[:], rb_ps[:, 0:1], AF.Identity, bias=bc2_s[:, 0:1])
        cb_ps = ptile([128, 2], "new", "cb_ps")
        for h in range(2):
            nc.tensor.matmul(cb_ps[:, h:h + 1], w1b_s[:, h * 128:(h + 1) * 128],
                             rb_s[:], start=True, stop=True)
        for h in range(2):
            nc.scalar.activation(cbias_s[:, h:h + 1], cb_ps[:, h:h + 1], AF.Identity,
                                 bias=b1_s[:, h:h + 1])

        # ---- evolve: 8 CA steps ----
        def emit_chunk(ci, s):
            c0 = ci * CH
            pre = [ptile([128, CH], "pre", f"pre{h_}") for h_ in range(2)]
            for h in range(2):
                hcols = slice(h * 128, (h + 1) * 128)
                for k in range(2):
                    d0 = c0 + k * 512
                    segs = [(wc_s, [(d0, d0, 512)]),
                            (wl_s, _pieces(d0, 512, -256)),
                            (wr_s, _pieces(d0, 512, +256))]
                    flat = [(w, dd, ss, ll) for w, ps in segs for dd, ss, ll in ps]
                    for i, (w, dd, ss, ll) in enumerate(flat):
                        nc.tensor.matmul(
                            pre[h][:, dd - c0: dd - c0 + ll],
                            w[:, hcols], sig[:, ss:ss + ll],
                            start=(i == 0), stop=(i == len(flat) - 1))
            h_t = [sbh.tile([128, CH], F32R, tag="h", name=f"ht{h_}")
                   for h_ in range(2)]
            for h in range(2):
                nc.scalar.activation(h_t[h][:], pre[h][:], AF.Gelu,
                                     bias=cbias_s[:, h:h + 1], scale=ia)
            new_ps = ptile([128, CH], "new", "new_ps")
            for k in range(2):
                jc = slice(k * 512, (k + 1) * 512)
                nc.tensor.matmul(new_ps[:, jc], w2_s[:, 0:128], h_t[0][:, jc],
                                 start=True, stop=False)
                nc.tensor.matmul(new_ps[:, jc], w2_s[:, 128:256], h_t[1][:, jc],
                                 start=False, stop=True)
            t_t = sbt.tile([128, CH], F32, tag="t", name="t_t")
            nc.scalar.activation(t_t[:], new_ps[:], AF.Tanh, bias=b2_s[:, 0:1])
            return t_t

        def emit_blend(ci, t_t):
            c0 = ci * CH
            nc.vector.scalar_tensor_tensor(
                sig[:, c0:c0 + CH], sig[:, c0:c0 + CH], a, t_t[:],
                op0=ALU.mult, op1=ALU.add)

        def emit_stats(ci):
            c0 = ci * CH
            sq_t = sbh.tile([128, CH], F32R, tag="h", name="sq_t")
            nc.scalar.activation(sq_t[:], sig[:, c0:c0 + CH], AF.Square)
            sr1 = ptile([1, CH], "pre", "sr1")
            sr2 = ptile([1, CH], "pre", "sr2")
            for k in range(2):
                jc = slice(k * 512, (k + 1) * 512)
                nc.tensor.matmul(sr1[0:1, jc], ones_s[:, 0:1],
                                 sig[:, c0 + k * 512:c0 + (k + 1) * 512],
                                 start=True, stop=True)
                nc.tensor.matmul(sr2[0:1, jc], ones_s[:, 0:1], sq_t[:, jc],
                                 start=True, stop=True)
            srow_t = sbsr.tile([1, 2 * CH], F32, tag="srow", name="srow_t")
            nc.scalar.activation(srow_t[0:1, 0:CH], sr1[0:1, :], AF.Copy)
            nc.vector.tensor_copy(srow_t[0:1, CH:2 * CH], sr2[0:1, :])
            stp = ptile([128, 16], "new", "stp")
            for j in range(CH // 128):
                nc.tensor.transpose(stp[:, 2 * j:2 * j + 1],
                                    srow_t[0:1, j * 128:(j + 1) * 128],
                                    eye2_s[0:1, 0:1])
                nc.tensor.transpose(stp[:, 2 * j + 1:2 * j + 2],
                                    srow_t[0:1, CH + j * 128:CH + (j + 1) * 128],
                                    eye2_s[0:1, 0:1])
            nc.vector.tensor_copy(stats_tm[:, 16 * ci:16 * (ci + 1)], stp[:])

        for s in range(NEV - 1):
            order = [(s + j) % NCH for j in range(NCH)]
            t_tiles = {}
            for ci in order:
                t_tiles[ci] = emit_chunk(ci, s)
            for ci in order:
                emit_blend(ci, t_tiles[ci])

        # last step: blends lag chunk processing by 2 (a blend only needs its
        # own and both neighbor chunks' matmuls done); each chunk's LN stats
        # follow its blend immediately, overlapping the rest of the step
        P = [(NCH - 2 + j) % NCH for j in range(NCH)]
        t7 = {}
        warm_s = sbst.tile([1, 8], F32, tag="warm")
        nc.scalar.activation(warm_s[:], cpack[0:1, 0:8], AF.Sqrt)
        for i, ci in enumerate(P):
            t7[ci] = emit_chunk(ci, NEV - 1)
            if i >= 2:
                emit_blend(P[i - 1], t7[P[i - 1]])
                emit_stats(P[i - 1])
        for ci in (P[7], P[0]):
            emit_blend(ci, t7[ci])
            emit_stats(ci)

        # ---- final: per-token inv-std + head ----
        st3 = stats_tm[:].rearrange("p (b two) -> p b two", two=2)
        s1ap = st3[:, :, 0]
        s2ap = st3[:, :, 1]
        m2_s = sbst.tile([128, NBLK], F32, tag="m2")
        nc.scalar.activation(m2_s[:], s1ap, AF.Square, scale=ia / 128.0)
        vf_s = sbst.tile([128, NBLK], F32, tag="vf")
        nc.vector.scalar_tensor_tensor(vf_s[:], s2ap, ia * ia / 128.0, m2_s[:],
                                       op0=ALU.mult, op1=ALU.subtract)
        nc.vector.tensor_scalar_add(vf_s[:], vf_s[:], EPS)
        sd_s = sbst.tile([128, NBLK], F32, tag="sd")
        nc.scalar.activation(sd_s[:], vf_s[:], AF.Sqrt)
        y0_s = sbst.tile([128, NBLK], F32, tag="y0")
        nc.vector.reciprocal(y0_s[:], sd_s[:])
        q_s = sbst.tile([128, NBLK], F32, tag="q")
        nc.vector.tensor_mul(q_s[:], y0_s[:], y0_s[:])
        w_s = sbst.tile([128, NBLK], F32, tag="w")
        nc.vector.scalar_tensor_tensor(w_s[:], vf_s[:], -0.5, q_s[:],
                                       op0=ALU.mult, op1=ALU.mult)
        inv_s = sbst.tile([128, NBLK], F32, tag="inv")
        nc.vector.scalar_tensor_tensor(inv_s[:], w_s[:], 1.5, y0_s[:],
                                       op0=ALU.add, op1=ALU.mult)
        # keep the PE busy through the col-math window (independent matmuls)
        for dwi in range(6):
            dummy_ps = ptile([128, 512], "new", "dummy_ps")
            nc.tensor.matmul(dummy_ps[:], wc_s[:, 0:128],
                             sig[:, dwi * 512:(dwi + 1) * 512], start=True, stop=True)
        # head: out[tok, v] = inv[tok] * (sigma_blk^T @ hwc)
        for b in range(NBLK):
            A_ps = ptile([128, V], "pre", "A_ps")
            nc.tensor.matmul(A_ps[:], sig[:, b * 128:(b + 1) * 128], hwc_s[:],
                             start=True, stop=True)
            o_t = sbo.tile([128, V], F32, tag="o", name="o_t")
            if b % 2 == 0:
                nc.vector.tensor_scalar(o_t[:], A_ps[:], inv_s[:, b:b + 1], None,
                                        ALU.mult)
            else:
                nc.scalar.activation(o_t[:], A_ps[:], AF.Copy,
                                     scale=inv_s[:, b:b + 1])
            tt = b // 2
            b0 = (b % 2) * 128
            nc.sync.dma_start(out_r[b0:b0 + 128, tt, :], o_t[:])

    nc.compile()
    return nc


def kernel(**inputs):
    g = {k: np.asarray(v, np.float32) if k != "tokens" else np.asarray(v)
         for k, v in inputs.items()}
    alpha = float(g["alpha"])
    a = float(1.0 / (1.0 + np.exp(-np.float64(alpha))))
    ia = 1.0 - a
    ln_b = g["ln_b"]
    has_lnb = bool(np.any(ln_b != 0))
    key = (np.float64(a).tobytes(), has_lnb)
    if key not in _CACHE:
        _CACHE[key] = _build(a, has_lnb)
    nc = _CACHE[key]

    W1, W2 = g["W1"], g["W2"]
    embed, pos = g["embed"], g["pos_embed"]
    head_w, ln_g = g["head_w"], g["ln_g"]

    spack = np.ones((128, 128), np.float32)

    cpack = np.zeros((128, 46), np.float32)
    cpack[:, 0:32] = pos.T * np.float32(1.0 / ia)
    cpack[:, 32:36] = g["c_states"].T
    cpack[:, 36:38] = g["bc1"].reshape(2, 128).T
    cpack[:, 38:39] = g["bc2"].reshape(128, 1)
    cpack[:, 39:41] = g["b1"].reshape(2, 128).T
    cpack[:, 41:42] = g["b2"].reshape(128, 1)
    cpack[:, 42:44] = np.stack([np.arange(128), np.arange(128, 256)], axis=1)
    cpack[0:2, 44:46] = np.eye(2, dtype=np.float32)

    epack = np.concatenate([embed[0:128], embed[128:256]],
                           axis=1) * np.float32(1.0 / ia)
    wpack = np.zeros((128, 1280), np.float32)
    wpack[:, 0:256] = W1[0:128]
    wpack[:, 256:512] = W1[128:256]
    wpack[:, 512:768] = W1[256:384]
    wpack[:, 768:1024] = np.concatenate([W2[0:128], W2[128:256]], axis=1)
    ghw = head_w * ln_g[:, None]
    wpack[:, 1024:1280] = (ghw - ghw.mean(axis=0, keepdims=True)) * np.float32(ia)

    fpack = np.zeros((128, 1024), np.float32)
    fpack[:, 0:256] = W1[384:512]
    fpack[:, 256:512] = g["Wc1"]
    fpack[:, 512:768] = np.concatenate([g["Wc2"][0:128], g["Wc2"][128:256]], axis=1)
    if has_lnb:
        fpack[0:1, 768:1024] = (ln_b @ head_w).reshape(1, V)

    tokens = g["tokens"]
    in_maps = []
    for c in range(NC):
        tk = tokens[c * BL:(c + 1) * BL].astype(np.float32)   # (BL, T)
        in_maps.append({
            "tok": np.ascontiguousarray(tk.T).reshape(1, NTOK),  # t-major
            "spack": spack, "cpack": cpack, "epack": epack,
            "wpack": wpack, "fpack": fpack,
        })

    kw = {}
    if TRACE:
        kw = dict(trace=True)
    res = run_bass_kernel_spmd(nc, in_maps, core_ids=list(range(NC)), **kw)
    if TRACE and res.exec_time_ns is not None:
        print(f"HW exec time: {res.exec_time_ns} ns")
        kernel.last_exec_ns = res.exec_time_ns
        kernel.last_trace = res.instructions_and_trace
    out = np.stack([res.results[c]["out"] for c in range(NC)], axis=0)
    out = out.reshape(B, T, V)
    if has_lnb:
        out = out + (ln_b @ head_w)[None, None, :]
    return np.ascontiguousarray(out)


# revision 26
# speedup vs baseline: 1.0928x; 1.0417x over previous
"""Trainium2 Bass kernel for nn_CellularAutomatonDecoder.

Model (per reference):
  cells = embed[tokens] + pos_embed                        (B, T, D)
  rule_bias MLP from mean(c_states); const_bias = rule_bias @ W1b + b1
  8x CA steps: pre = cells@W1c + roll(cells,+1)@W1l + roll(cells,-1)@W1r + const_bias
               cells = a*cells + (1-a)*tanh(gelu(pre) @ W2 + b2)
  out = LN(cells) @ head_w                                 (B, T, V)

Sharding: pure data-parallel over batch across 8 cores (256 rows each).

Device design notes:
- feature-major state sigma[d=128 partitions, 8192 tokens] in SBUF, token
  order t-major (col j = t*256 + b_local): the T-axis roll becomes a +-256
  column shift with one wrap piece -> every matmul is contiguous with
  N>=256, where fp32r streams at ~1 cycle/row.
- state kept scaled: sigma = cells/(1-a); the leaky blend is one fused DVE
  op sigma' = a*sigma + tanh(...); (1-a) folds into activation scales and
  host-prescaled weights.
- embedding gather = one-hot matmul (one-hot built on DVE from a K=1
  token-broadcast matmul + iota-column compare).
- head uses sigma blocks as the stationary operand so output lands
  token-major in PSUM and DMAs out contiguously; LayerNorm reduces to a
  per-token inv-std scale (ln_g and mean-removal folded into a
  column-centered head weight matrix on the host).
- LN stats via ones-matmuls -> PE micro-transposes -> token-major [128,64]
  tiles, so all per-token scalar math runs across partitions.
- single PSUM pool with shared tags across init/evolve/final: no pool
  release barriers between phases (keeps PE dense, avoids HAM re-throttle).
"""

import os
import sys

import numpy as np

for _p in ("/opt/trn_rl_repo", "/root/.axon_site/_ro/trn_rl_repo"):
    if os.path.isdir(_p) and _p not in sys.path:
        sys.path.append(_p)

from contextlib import ExitStack

import concourse.bacc as bacc
import concourse.tile as tile
from concourse import mybir
from concourse.bass_utils import run_bass_kernel_spmd

F32 = mybir.dt.float32
F32R = mybir.dt.float32r
AF = mybir.ActivationFunctionType
ALU = mybir.AluOpType
AX = mybir.AxisListType

B, T, D, V, CDIM = 2048, 32, 128, 256, 128
NEV = 8
EPS = 1e-5
NC = 8
BL = B // NC          # 256 batch rows per core
NTOK = BL * T         # 8192 tokens per core
CH = 1024             # token chunk (columns)
NCH = NTOK // CH      # 8 chunks
NBLK = NTOK // 128    # 64 head blocks

TRACE = False         # test harness may flip this (with prof shim installed)
_CACHE = {}


def _pieces(dst0, n, shift):
    """Contiguous (dst, src, len) pieces of src = (dst + shift) mod NTOK."""
    out = []
    j = 0
    while j < n:
        s = (dst0 + j + shift) % NTOK
        ln = min(n - j, NTOK - s)
        out.append((dst0 + j, s, ln))
        j += ln
    return out


def _build(a, has_lnb):
    ia = 1.0 - a
    nc = bacc.Bacc("TRN2", target_bir_lowering=False, debug=False, num_devices=NC)

    tok_d = nc.dram_tensor("tok", [1, NTOK], F32R, kind="ExternalInput").ap()
    spack_d = nc.dram_tensor("spack", [128, 128], F32R, kind="ExternalInput").ap()
    cpack_d = nc.dram_tensor("cpack", [128, 46], F32, kind="ExternalInput").ap()
    epack_d = nc.dram_tensor("epack", [128, 256], F32R, kind="ExternalInput").ap()
    wpack_d = nc.dram_tensor("wpack", [128, 1280], F32R, kind="ExternalInput").ap()
    fpack_d = nc.dram_tensor("fpack", [128, 1024], F32, kind="ExternalInput").ap()
    out_d = nc.dram_tensor("out", [NTOK, V], F32, kind="ExternalOutput").ap()
    out_r = out_d.rearrange("(b t) v -> b t v", t=T)

    with tile.TileContext(nc) as tc, ExitStack() as ctx:
        # ---- persistent SBUF ----
        wpool = ctx.enter_context(tc.tile_pool(name="weights", bufs=1))
        spack = wpool.tile([128, 128], F32R, tag="spack")
        nc.sync.dma_start(spack[:], spack_d)
        cpack = wpool.tile([128, 46], F32, tag="cpack")
        nc.sync.dma_start(cpack[:], cpack_d)
        epack = wpool.tile([128, 256], F32R, tag="epack")
        nc.sync.dma_start(epack[:], epack_d)
        wpack = wpool.tile([128, 1280], F32R, tag="wpack")
        nc.sync.dma_start(wpack[:], wpack_d)
        fpack = wpool.tile([128, 1024], F32, tag="fpack")
        nc.sync.dma_start(fpack[:], fpack_d)

        ones_s = spack
        emb_s = epack[:, 0:256]
        wc_s, wl_s, wr_s = wpack[:, 0:256], wpack[:, 256:512], wpack[:, 512:768]
        w2_s, hwc_s = wpack[:, 768:1024], wpack[:, 1024:1280]
        w1b_s, wc1_s, wc2_s = fpack[:, 0:256], fpack[:, 256:512], fpack[:, 512:768]
        consth_r = fpack[0:1, 768:1024].bitcast(F32R)
        posT_s, cT_s = cpack[:, 0:32], cpack[:, 32:36]
        bc1_s, bc2_s = cpack[:, 36:38], cpack[:, 38:39]
        b1_s, b2_s = cpack[:, 39:41], cpack[:, 41:42]
        vid_s = cpack[:, 42:44]
        eye2_s = cpack[0:2, 44:46]

        spool = ctx.enter_context(tc.tile_pool(name="state", bufs=1))
        sig = spool.tile([128, NTOK], F32R, tag="sigma")
        stats_tm = spool.tile([128, 2 * NBLK], F32, tag="stats_tm")

        mlp_sb = ctx.enter_context(tc.tile_pool(name="mlp_sb", bufs=1))
        cbias_s = mlp_sb.tile([128, 2], F32, tag="cbias")

        # shared pools, all phases (no release barriers)
        pp = ctx.enter_context(tc.tile_pool(name="psum", bufs=1, space="PSUM"))
        # tag "pre": 3 slots x [128,1024] (2 banks each); tag "new": 1 slot
        sbh = ctx.enter_context(tc.tile_pool(name="h_sb", bufs=4))
        sbt = ctx.enter_context(tc.tile_pool(name="t_sb", bufs=NCH + 1))
        sbtok = ctx.enter_context(tc.tile_pool(name="tok_sb", bufs=2))
        sbsr = ctx.enter_context(tc.tile_pool(name="srow_sb", bufs=3))
        sbst = ctx.enter_context(tc.tile_pool(name="stat_sb", bufs=1))
        sbo = ctx.enter_context(tc.tile_pool(name="out_sb", bufs=4))

        def ptile(shape, tag, name):
            return pp.tile(shape, F32, tag=tag, name=name, bufs=3 if tag == "pre" else 1)

        # ---- init: token gather via one-hot matmuls ----
        # tokens broadcast to all partitions by DMA (gpsimd queue, overlaps
        # the weight DMAs); the one-hot compares then run from SBUF and the
        # 16 K=1 broadcast matmuls disappear from the (cold) PE stream
        grp_tiles = {}
        for g in (3, 0, 1, 2):
            tb = sbtok.tile([128, 2 * CH], F32R, tag="tok", name="tokb")
            src = tok_d[0:1, g * 2 * CH:(g + 1) * 2 * CH].broadcast_to((128, 2 * CH))
            nc.gpsimd.dma_start(tb[:], src)
            grp_tiles[g] = tb
        for ci in [6, 7, 0, 1, 2, 3, 4, 5]:
            c0 = ci * CH
            tb = grp_tiles[ci // 2]
            tsl = tb[:, (ci % 2) * CH:(ci % 2 + 1) * CH]
            oh_lo = sbh.tile([128, CH], F32R, tag="h", name="oh_lo")
            oh_hi = sbh.tile([128, CH], F32R, tag="h", name="oh_hi")
            nc.vector.tensor_scalar(oh_lo[:], tsl, vid_s[:, 0:1], None, ALU.is_equal)
            nc.vector.tensor_scalar(oh_hi[:], tsl, vid_s[:, 1:2], None, ALU.is_equal)
            cells_ps = ptile([128, CH], "pre", "cells_ps")
            for k in range(2):
                jc = slice(k * 512, (k + 1) * 512)
                nc.tensor.matmul(cells_ps[:, jc], emb_s[:, 0:128], oh_lo[:, jc],
                                 start=True, stop=False)
                nc.tensor.matmul(cells_ps[:, jc], emb_s[:, 128:256], oh_hi[:, jc],
                                 start=False, stop=True)
            for kb in range(CH // 256):
                tt = (c0 + kb * 256) // 256  # col j = t*256 + b -> t = j//256
                nc.scalar.activation(sig[:, c0 + kb * 256: c0 + (kb + 1) * 256],
                                     cells_ps[:, kb * 256:(kb + 1) * 256],
                                     AF.Identity, bias=posT_s[:, tt:tt + 1])

        # ---- rule-bias MLP (tiny; overlaps gather) ----
        cp_s = mlp_sb.tile([128, 1], F32, tag="cp")
        nc.vector.tensor_reduce(cp_s[:], cT_s[:], axis=AX.X, op=ALU.add)
        y1_ps = ptile([128, 2], "new", "y1_ps")
        for h in range(2):
            nc.tensor.matmul(y1_ps[:, h:h + 1], wc1_s[:, h * 128:(h + 1) * 128],
                             cp_s[:], start=True, stop=True)
        y1g_s = mlp_sb.tile([128, 2], F32, tag="y1g")
        for h in range(2):
            nc.scalar.activation(y1g_s[:, h:h + 1], y1_ps[:, h:h + 1], AF.Gelu,
                                 bias=bc1_s[:, h:h + 1], scale=0.25)
        rb_ps = ptile([128, 2], "new", "rb_ps")
        nc.tensor.matmul(rb_ps[:, 0:1], wc2_s[:, 0:128], y1g_s[:, 0:1],
                         start=True, stop=False)
        nc.tensor.matmul(rb_ps[:, 0:1], wc2_s[:, 128:256], y1g_s[:, 1:2],
                         start=False, stop=True)
        rb_s = mlp_sb.tile([128, 1], F32, tag="rb")
        nc.scalar.activation(rb_s[:], rb_ps[:, 0:1], AF.Identity, bias=bc2_s[:, 0:1])
        cb_ps = ptile([128, 2], "new", "cb_ps")
        for h in range(2):
            nc.tensor.matmul(cb_ps[:, h:h + 1], w1b_s[:, h * 128:(h + 1) * 128],
                             rb_s[:], start=True, stop=True)
        for h in range(2):
            nc.scalar.activation(cbias_s[:, h:h + 1], cb_ps[:, h:h + 1], AF.Identity,
                                 bias=b1_s[:, h:h + 1])

        # ---- evolve: 8 CA steps ----
        def emit_chunk(ci, s):
            c0 = ci * CH
            pre = [ptile([128, CH], "pre", f"pre{h_}") for h_ in range(2)]
            for h in range(2):
                hcols = slice(h * 128, (h + 1) * 128)
                for k in range(2):
                    d0 = c0 + k * 512
                    segs = [(wc_s, [(d0, d0, 512)]),
                            (wl_s, _pieces(d0, 512, -256)),
                            (wr_s, _pieces(d0, 512, +256))]
                    flat = [(w, dd, ss, ll) for w, ps in segs for dd, ss, ll in ps]
                    for i, (w, dd, ss, ll) in enumerate(flat):
                        nc.tensor.matmul(
                            pre[h][:, dd - c0: dd - c0 + ll],
                            w[:, hcols], sig[:, ss:ss + ll],
                            start=(i == 0), stop=(i == len(flat) - 1))
            h_t = [sbh.tile([128, CH], F32R, tag="h", name=f"ht{h_}")
                   for h_ in range(2)]
            for h in range(2):
                nc.scalar.activation(h_t[h][:], pre[h][:], AF.Gelu,
                                     bias=cbias_s[:, h:h + 1], scale=ia)
            new_ps = ptile([128, CH], "new", "new_ps")
            for k in range(2):
                jc = slice(k * 512, (k + 1) * 512)
                nc.tensor.matmul(new_ps[:, jc], w2_s[:, 0:128], h_t[0][:, jc],
                                 start=True, stop=False)
                nc.tensor.matmul(new_ps[:, jc], w2_s[:, 128:256], h_t[1][:, jc],
                                 start=False, stop=True)
            t_t = sbt.tile([128, CH], F32, tag="t", name="t_t")
            nc.scalar.activation(t_t[:], new_ps[:], AF.Tanh, bias=b2_s[:, 0:1])
            return t_t

        def emit_blend(ci, t_t):
            c0 = ci * CH
            nc.vector.scalar_tensor_tensor(
                sig[:, c0:c0 + CH], sig[:, c0:c0 + CH], a, t_t[:],
                op0=ALU.mult, op1=ALU.add)

        def emit_stats(ci):
            c0 = ci * CH
            sq_t = sbh.tile([128, CH], F32R, tag="h", name="sq_t")
            nc.scalar.activation(sq_t[:], sig[:, c0:c0 + CH], AF.Square)
            sr1 = ptile([1, CH], "pre", "sr1")
            sr2 = ptile([1, CH], "pre", "sr2")
            for k in range(2):
                jc = slice(k * 512, (k + 1) * 512)
                nc.tensor.matmul(sr1[0:1, jc], ones_s[:, 0:1],
                                 sig[:, c0 + k * 512:c0 + (k + 1) * 512],
                                 start=True, stop=True)
                nc.tensor.matmul(sr2[0:1, jc], ones_s[:, 0:1], sq_t[:, jc],
                                 start=True, stop=True)
            srow_t = sbsr.tile([1, 2 * CH], F32, tag="srow", name="srow_t")
            nc.scalar.activation(srow_t[0:1, 0:CH], sr1[0:1, :], AF.Copy)
            nc.vector.tensor_copy(srow_t[0:1, CH:2 * CH], sr2[0:1, :])
            stp = ptile([128, 16], "new", "stp")
            for j in range(CH // 128):
                nc.tensor.transpose(stp[:, 2 * j:2 * j + 1],
                                    srow_t[0:1, j * 128:(j + 1) * 128],
                                    eye2_s[0:1, 0:1])
                nc.tensor.transpose(stp[:, 2 * j + 1:2 * j + 2],
                                    srow_t[0:1, CH + j * 128:CH + (j + 1) * 128],
                                    eye2_s[0:1, 0:1])
            nc.vector.tensor_copy(stats_tm[:, 16 * ci:16 * (ci + 1)], stp[:])

        for s in range(NEV - 1):
            order = [(s + j) % NCH for j in range(NCH)]
            t_tiles = {}
            for ci in order:
                t_tiles[ci] = emit_chunk(ci, s)
            for ci in order:
                emit_blend(ci, t_tiles[ci])

        # last step: blends lag chunk processing by 2 (a blend only needs its
        # own and both neighbor chunks' matmuls done); each chunk's LN stats
        # follow its blend immediately, overlapping the rest of the step
        P = [(NCH - 2 + j) % NCH for j in range(NCH)]
        t7 = {}
        warm_s = sbst.tile([1, 8], F32, tag="warm")
        nc.scalar.activation(warm_s[:], cpack[0:1, 0:8], AF.Sqrt)
        for i, ci in enumerate(P):
            t7[ci] = emit_chunk(ci, NEV - 1)
            if i >= 2:
                emit_blend(P[i - 1], t7[P[i - 1]])
                emit_stats(P[i - 1])
        for ci in (P[7], P[0]):
            emit_blend(ci, t7[ci])
            emit_stats(ci)

        # ---- final: per-token inv-std + head ----
        st3 = stats_tm[:].rearrange("p (b two) -> p b two", two=2)
        s1ap = st3[:, :, 0]
        s2ap = st3[:, :, 1]
        m2_s = sbst.tile([128, NBLK], F32, tag="m2")
        nc.scalar.activation(m2_s[:], s1ap, AF.Square, scale=ia / 128.0)
        vf_s = sbst.tile([128, NBLK], F32, tag="vf")
        nc.vector.scalar_tensor_tensor(vf_s[:], s2ap, ia * ia / 128.0, m2_s[:],
                                       op0=ALU.mult, op1=ALU.subtract)
        nc.vector.tensor_scalar_add(vf_s[:], vf_s[:], EPS)
        sd_s = sbst.tile([128, NBLK], F32, tag="sd")
        nc.scalar.activation(sd_s[:], vf_s[:], AF.Sqrt)
        y0_s = sbst.tile([128, NBLK], F32, tag="y0")
        nc.vector.reciprocal(y0_s[:], sd_s[:])
        q_s = sbst.tile([128, NBLK], F32, tag="q")
        nc.vector.tensor_mul(q_s[:], y0_s[:], y0_s[:])
        w_s = sbst.tile([128, NBLK], F32, tag="w")
        nc.vector.scalar_tensor_tensor(w_s[:], vf_s[:], -0.5, q_s[:],
                                       op0=ALU.mult, op1=ALU.mult)
        inv_s = sbst.tile([128, NBLK], F32, tag="inv")
        nc.vector.scalar_tensor_tensor(inv_s[:], w_s[:], 1.5, y0_s[:],
                                       op0=ALU.add, op1=ALU.mult)
        # keep the PE busy through the col-math window (independent matmuls)
        for dwi in range(6):
            dummy_ps = ptile([128, 512], "new", "dummy_ps")
            nc.tensor.matmul(dummy_ps[:], wc_s[:, 0:128],
                             sig[:, dwi * 512:(dwi + 1) * 512], start=True, stop=True)
        # head: out[tok, v] = inv[tok] * (sigma_blk^T @ hwc)
        for b in range(NBLK):
            A_ps = ptile([128, V], "pre", "A_ps")
            nc.tensor.matmul(A_ps[:], sig[:, b * 128:(b + 1) * 128], hwc_s[:],
                             start=True, stop=True)
            o_t = sbo.tile([128, V], F32, tag="o", name="o_t")
            if b % 2 == 0:
                nc.vector.tensor_scalar(o_t[:], A_ps[:], inv_s[:, b:b + 1], None,
                                        ALU.mult)
            else:
                nc.scalar.activation(o_t[:], A_ps[:], AF.Copy,
                                     scale=inv_s[:, b:b + 1])
            tt = b // 2
            b0 = (b % 2) * 128
            nc.sync.dma_start(out_r[b0:b0 + 128, tt, :], o_t[:])

    nc.compile()
    return nc


def kernel(**inputs):
    g = {k: np.asarray(v, np.float32) if k != "tokens" else np.asarray(v)
         for k, v in inputs.items()}
    alpha = float(g["alpha"])
    a = float(1.0 / (1.0 + np.exp(-np.float64(alpha))))
    ia = 1.0 - a
    ln_b = g["ln_b"]
    has_lnb = bool(np.any(ln_b != 0))
    key = (np.float64(a).tobytes(), has_lnb)
    if key not in _CACHE:
        _CACHE[key] = _build(a, has_lnb)
    nc = _CACHE[key]

    W1, W2 = g["W1"], g["W2"]
    embed, pos = g["embed"], g["pos_embed"]
    head_w, ln_g = g["head_w"], g["ln_g"]

    spack = np.ones((128, 128), np.float32)

    cpack = np.zeros((128, 46), np.float32)
    cpack[:, 0:32] = pos.T * np.float32(1.0 / ia)
    cpack[:, 32:36] = g["c_states"].T
    cpack[:, 36:38] = g["bc1"].reshape(2, 128).T
    cpack[:, 38:39] = g["bc2"].reshape(128, 1)
    cpack[:, 39:41] = g["b1"].reshape(2, 128).T
    cpack[:, 41:42] = g["b2"].reshape(128, 1)
    cpack[:, 42:44] = np.stack([np.arange(128), np.arange(128, 256)], axis=1)
    cpack[0:2, 44:46] = np.eye(2, dtype=np.float32)

    epack = np.concatenate([embed[0:128], embed[128:256]],
                           axis=1) * np.float32(1.0 / ia)
    wpack = np.zeros((128, 1280), np.float32)
    wpack[:, 0:256] = W1[0:128]
    wpack[:, 256:512] = W1[128:256]
    wpack[:, 512:768] = W1[256:384]
    wpack[:, 768:1024] = np.concatenate([W2[0:128], W2[128:256]], axis=1)
    ghw = head_w * ln_g[:, None]
    wpack[:, 1024:1280] = (ghw - ghw.mean(axis=0, keepdims=True)) * np.float32(ia)

    fpack = np.zeros((128, 1024), np.float32)
    fpack[:, 0:256] = W1[384:512]
    fpack[:, 256:512] = g["Wc1"]
    fpack[:, 512:768] = np.concatenate([g["Wc2"][0:128], g["Wc2"][128:256]], axis=1)
    if has_lnb:
        fpack[0:1, 768:1024] = (ln_b @ head_w).reshape(1, V)

    tokens = g["tokens"]
    in_maps = []
    for c in range(NC):
        tk = tokens[c * BL:(c + 1) * BL].astype(np.float32)   # (BL, T)
        in_maps.append({
            "tok": np.ascontiguousarray(tk.T).reshape(1, NTOK),  # t-major
            "spack": spack, "cpack": cpack, "epack": epack,
            "wpack": wpack, "fpack": fpack,
        })

    kw = {}
    if TRACE:
        kw = dict(trace=True)
    res = run_bass_kernel_spmd(nc, in_maps, core_ids=list(range(NC)), **kw)
    if TRACE and res.exec_time_ns is not None:
        print(f"HW exec time: {res.exec_time_ns} ns")
        kernel.last_exec_ns = res.exec_time_ns
        kernel.last_trace = res.instructions_and_trace
    out = np.stack([res.results[c]["out"] for c in range(NC)], axis=0)
    out = out.reshape(B, T, V)
    if has_lnb:
        out = out + (ln_b @ head_w)[None, None, :]
    return np.ascontiguousarray(out)
